# revision 14
# baseline (speedup 1.0000x reference)
"""Causal self-attention on 8 TRN2 NeuronCores.

Problem: B=4, S=2048, D=1024, H=16 heads (hd=64), fp32 in/out.
  qkv = x @ w_qkv + b_qkv ; causal softmax attention ; y @ w_out + b_out

Sharding (tensor-parallel over heads x data-parallel over batch):
  core c -> batch b = c//2, head-group hg = c%2 (8 heads each).
  Each core computes qkv for its 8 heads from x[b], runs attention, and
  produces a partial output  y_local @ w_out[rows]  of shape [S, D] (bf16).
  Host unshards: out[b] = partial[2b] + partial[2b+1] + b_out (fp32).

Device kernel (per core), bf16 matmul operands / fp32 PSUM accumulation:
  - Every DRAM input is laid out host-side exactly as its SBUF tile
    (partition-dim first), so every load is a contiguous max-rate DMA,
    and everything (weights for all 4 head-pairs + x) is prefetched at
    kernel start.
  - x arrives token-chunked ([TB,128,KO,512]) so the v projection starts
    as soon as the first chunk lands; v for token tiles 8..15 is woven
    into pr0's attention stream to keep the PE saturated.
  - q,k produced directly transposed (qT/kT [64,S] per head) via
    out = w.T @ x; heads processed in pairs packed at partition offsets
    0-63 / 64-127, so the two K=64 score matmuls occupy disjoint PE row
    halves and execute concurrently.  v in natural layout with a ones
    column (v_aug) so the PV matmul also produces the softmax
    denominator ("ones trick").
  - scores computed transposed (S_T[k,q]); the head pair's scores land
    in the two banks of one [128,2,512] PSUM tile so a single ACT exp
    instruction covers both heads.  Causal handled by block skipping;
    diagonal tiles get an additive -1e30 mask on only the first 128
    columns of the live strip (the rest is always valid).
  - softmax normalization is entirely off the ACT engine (no table
    swaps): denominator rows are evicted per q-block into an [8,512]
    SBUF tile, one DVE reciprocal_approx_fast per head-pair, gpsimd
    partition_broadcast, then an in-place DVE multiply on the bf16 yT.
  - emission is software-pipelined: score matmuls lead PV matmuls by 2
    steps so the in-order PE queue never head-of-line blocks on exp.
"""

import os
import sys

for _p in ("/root/.axon_site/_ro/trn_rl_repo", "/opt/trn_rl_repo"):
    if os.path.isdir(_p) and _p not in sys.path:
        sys.path.append(_p)

import ml_dtypes
import numpy as np

import concourse.bass as bass  # noqa: F401
import concourse.mybir as mybir
import concourse.tile as tile
from concourse import bacc
from concourse.bass_utils import run_bass_kernel_spmd

B, S, D, H = 4, 2048, 1024, 16
HD = 64
HPC = 8          # heads per core
NPAIR = HPC // 2
KO = D // 128    # contraction chunks over D
ATT_SCALE = 1.0 / np.sqrt(HD)
NEG = -1.0e30

F32 = mybir.dt.float32
BF16 = mybir.dt.bfloat16
NPBF16 = ml_dtypes.bfloat16


def build_nc(S_=S):
    KT = S_ // 128    # k tiles
    TB = S_ // 512    # token chunks
    NA = S_ // 512    # q blocks

    nc = bacc.Bacc(None)
    x_d = nc.dram_tensor("x", [TB, 128, KO, 512], BF16, kind="ExternalInput")
    wqk_d = nc.dram_tensor("wqk", [128, KO, NPAIR, 2, 128], BF16, kind="ExternalInput")
    bqk_d = nc.dram_tensor("bqk", [128, NPAIR, 2], F32, kind="ExternalInput")
    wv_d = nc.dram_tensor("wv", [128, KO, 512], BF16, kind="ExternalInput")
    bv_d = nc.dram_tensor("bv", [128, 512], F32, kind="ExternalInput")
    wout_d = nc.dram_tensor("wout", [128, NPAIR, D], BF16, kind="ExternalInput")
    mask_d = nc.dram_tensor("mask", [128, 128], F32, kind="ExternalInput")
    out_d = nc.dram_tensor("out", [S_, D], BF16, kind="ExternalOutput")

    with tile.TileContext(nc) as tc, nc.allow_low_precision("bf16 matmul operands / bf16 partial outputs"):
        with (
            tc.tile_pool(name="const", bufs=1) as constp,
            tc.tile_pool(name="psS", bufs=2, space="PSUM") as psS,
            tc.tile_pool(name="psY", bufs=2, space="PSUM") as psY,
            tc.tile_pool(name="patt", bufs=4) as patt,
            tc.tile_pool(name="pqk", bufs=2) as pqk,
            tc.tile_pool(name="pnorm", bufs=2) as pnorm,
            tc.tile_pool(name="pden", bufs=2) as pden,
            tc.tile_pool(name="postage", bufs=3) as postage,
        ):
            # ---- everything prefetched; DRAM layouts match SBUF tiles.
            # wv + x chunk 0 first (v-proj's first dependency); triggers
            # spread across engine queues so transfers parallelize early ----
            wv_sb = constp.tile([128, KO, 512], BF16)
            nc.sync.dma_start(wv_sb[:], wv_d[:])
            xT = constp.tile([128, TB, KO, 512], BF16)
            nc.scalar.dma_start(xT[:, 0], x_d[0])
            nc.gpsimd.dma_start(xT[:, 1], x_d[1])
            nc.scalar.dma_start(xT[:, 2], x_d[2])
            bv_sb = constp.tile([128, 512], F32)
            nc.sync.dma_start(bv_sb[:], bv_d[:])
            mask_sb = constp.tile([128, 128], F32)
            nc.sync.dma_start(mask_sb[:], mask_d[:])
            bqk_sb = constp.tile([128, NPAIR, 2], F32)
            nc.sync.dma_start(bqk_sb[:], bqk_d[:])
            nc.sync.dma_start(xT[:, 3], x_d[3])
            wqk_sb = constp.tile([128, KO, NPAIR, 2, 128], BF16)
            nc.sync.dma_start(wqk_sb[:], wqk_d[:])
            wout_sb = constp.tile([128, NPAIR, D], BF16)
            nc.sync.dma_start(wout_sb[:], wout_d[:])

            vaug = constp.tile([128, KT, HPC, 66], BF16)
            nc.gpsimd.memset(vaug[:, :, :, 64], 1.0)
            yT = constp.tile([128, NPAIR, S_], BF16)

            # ---- v projection for token-tile pair (2tp, 2tp+1) ----
            def emit_vproj(tp):
                ps = psS.tile([128, 2, 512], F32, tag="ps", name=f"psv{tp % 2}")
                for k in range(KO):
                    for i in range(2):
                        tt = 2 * tp + i
                        nc.tensor.matmul(
                            ps[:, i, :],
                            xT[:, tt // 4, k, (tt % 4) * 128 : (tt % 4) * 128 + 128],
                            wv_sb[:, k, :],
                            start=(k == 0),
                            stop=(k == KO - 1),
                        )
                for i in range(2):
                    tt = 2 * tp + i
                    nc.vector.tensor_tensor(
                        vaug[:, tt, :, 0:64],
                        ps[:, i, :].rearrange("p (h d) -> p h d", h=HPC),
                        bv_sb[:].rearrange("p (h d) -> p h d", h=HPC),
                        mybir.AluOpType.add,
                    )

            # ---- q/k projection for head pair pr, packed 64|64 ----
            def qk_group(pr, dst, cqk, tp, holder, k):
                if k == 0:
                    holder["ps"] = psS.tile(
                        [128, 2, 512], F32, tag="ps", name=f"psqk{pr}_{cqk}_{tp}"
                    )
                ps = holder["ps"]
                for i in range(2):
                    nc.tensor.matmul(
                        ps[:, i, :],
                        wqk_sb[:, k, pr, cqk, :],
                        xT[:, 2 * tp + i, k, :],
                        start=(k == 0),
                        stop=(k == KO - 1),
                    )
                if k == KO - 1:
                    nc.vector.tensor_scalar_add(
                        dst[:, tp * 1024 : (tp + 1) * 1024].rearrange(
                            "p (i t) -> p i t", i=2
                        ),
                        ps[:],
                        bqk_sb[:, pr, cqk : cqk + 1],
                    )

            def make_qk_closures(pr, qT, kT):
                cls = []
                for cqk in range(2):
                    dst = qT if cqk == 0 else kT
                    for tp in range(TB // 2):
                        holder = {}
                        for k in range(KO):
                            cls.append(
                                lambda pr=pr, dst=dst, cqk=cqk, tp=tp, holder=holder, k=k: qk_group(
                                    pr, dst, cqk, tp, holder, k
                                )
                            )
                return cls

            def emit_qk(pr, qT, kT):
                for c in make_qk_closures(pr, qT, kT):
                    c()

            # ---- output projection tile tt: partial = yT.T @ w_out (bf16) ----
            def out_group(tt, holder, part):
                if part == 0:
                    holder["ps"] = psS.tile(
                        [128, 2, 512], F32, tag="ps", name=f"pso{tt}"
                    )
                ps = holder["ps"]
                if part < 2:
                    nh = part
                    for cc in range(NPAIR):
                        nc.tensor.matmul(
                            ps[:, nh, :],
                            yT[:, cc, tt * 128 : (tt + 1) * 128],
                            wout_sb[:, cc, nh * 512 : (nh + 1) * 512],
                            start=(cc == 0),
                            stop=(cc == NPAIR - 1),
                        )
                else:
                    ot = postage.tile([128, D], BF16, tag="ot", name=f"ot{tt % 3}")
                    nc.vector.tensor_copy(
                        ot[:].rearrange("p (i t) -> p i t", i=2), ps[:]
                    )
                    nc.sync.dma_start(out_d[tt * 128 : (tt + 1) * 128, :], ot[:])

            def make_out_closures(tt):
                holder = {}
                return [
                    lambda tt=tt, holder=holder, part=part: out_group(tt, holder, part)
                    for part in range(3)
                ]

            qkt = {}
            out_done = [0]
            for pr in range(NPAIR):
                if pr == 0:
                    qkt[0] = (
                        pqk.tile([128, S_], BF16, tag="qT", name="qT0"),
                        pqk.tile([128, S_], BF16, tag="kT", name="kT0"),
                    )
                    for tp in range(4):
                        emit_vproj(tp)
                    emit_qk(0, *qkt[0])
                qT, kT = qkt[pr]
                # work to hide under this pr's ACT-bound attention stream:
                # the next pair's q/k projection, or (last pair) the output
                # projection for q-blocks whose normalization has completed
                pending = []
                if pr + 1 < NPAIR:
                    qkt[pr + 1] = (
                        pqk.tile([128, S_], BF16, tag="qT", name=f"qT{pr + 1}"),
                        pqk.tile([128, S_], BF16, tag="kT", name=f"kT{pr + 1}"),
                    )
                    pending = make_qk_closures(pr + 1, *qkt[pr + 1])

                steps = [(a, j) for a in range(NA) for j in range(4 * a + 4)]
                n_steps = len(steps)
                psy_by_a = {}
                att_by_idx = {}

                def emit_s(idx):
                    a, j = steps[idx]
                    o = 128 * j - 512 * a
                    oo = max(o, 0)
                    W = 512 - oo
                    pss = psS.tile([128, 2, 512], F32, tag="ps", name=f"pss{idx % 2}")
                    for h01 in range(2):
                        lo = h01 * 64
                        nc.tensor.matmul(
                            pss[:, h01, 0:W],
                            kT[lo : lo + 64, j * 128 : (j + 1) * 128],
                            qT[lo : lo + 64, a * 512 + oo : (a + 1) * 512],
                            start=True,
                            stop=True,
                        )
                    if o >= 0:
                        # only the first 128 columns of the live strip can
                        # violate causality within this k-tile
                        wm = min(W, 128)
                        for h01 in range(2):
                            nc.vector.tensor_tensor(
                                pss[:, h01, 0:wm],
                                pss[:, h01, 0:wm],
                                mask_sb[:, 0:wm],
                                mybir.AluOpType.add,
                            )
                    att = patt.tile([128, 2, 512], BF16, tag="att", name=f"att{idx % 4}")
                    nc.scalar.activation(
                        att[:, :, oo:512],
                        pss[:, :, 0:W],
                        mybir.ActivationFunctionType.Exp,
                        scale=float(ATT_SCALE),
                    )
                    att_by_idx[idx] = att

                def emit_pv(idx):
                    a, j = steps[idx]
                    nj = 4 * a + 4
                    oo = max(128 * j - 512 * a, 0)
                    if j == 0:
                        psy_by_a[a] = psY.tile(
                            [65, 2, 512], F32, tag="psy", name=f"psy{a % 2}"
                        )
                    psy = psy_by_a[a]
                    att = att_by_idx.pop(idx)
                    for h01 in range(2):
                        nc.tensor.matmul(
                            psy[:, h01, oo:512],
                            vaug[:, j, 2 * pr + h01, 0:65],
                            att[:, h01, oo:512],
                            start=(j == 0),
                            stop=(j == nj - 1),
                            skip_group_check=True,
                        )
                    if j == nj - 1:
                        dsts = a * 512
                        nc.vector.tensor_copy(
                            yT[0:64, pr, dsts : dsts + 512], psy[0:64, 0, :]
                        )
                        stg = pnorm.tile([64, 512], BF16, tag="stg")
                        nc.vector.tensor_copy(stg[:], psy[0:64, 1, :])
                        nc.sync.dma_start(yT[64:128, pr, dsts : dsts + 512], stg[:])
                        # DVE is lane-locked: stage the denominator row at
                        # partition 64, then DMA does the partition shift;
                        # reciprocal + broadcast + normalize all per-a so the
                        # chain overlaps the next q-block's attention stream
                        dstage = pnorm.tile([65, 2, 512], F32, tag="dstage")
                        nc.vector.tensor_copy(dstage[64:65, :, :], psy[64:65, :, :])
                        den2 = pden.tile([1, 2, 512], F32, tag="den2")
                        nc.sync.dma_start(den2[:], dstage[64:65, :, :])
                        nc.vector.reciprocal_approx_fast(den2[:], den2[:])
                        for h01 in range(2):
                            bc = pnorm.tile([128, 512], F32, tag="bc")
                            nc.gpsimd.partition_broadcast(bc[:], den2[0:1, h01, :])
                            rows = slice(64 * h01, 64 * h01 + 64)
                            nc.vector.tensor_tensor(
                                yT[rows, pr, dsts : dsts + 512],
                                yT[rows, pr, dsts : dsts + 512],
                                bc[rows, :],
                                mybir.AluOpType.mult,
                            )

                emit_s(0)
                emit_s(1)
                for idx in range(n_steps):
                    a, j = steps[idx]
                    if pr == 0 and j == 0 and a in (1, 2):
                        # weave v-proj for token tiles 8..15 into pr0's
                        # attention stream (needed first by a=2 / a=3)
                        emit_vproj(2 + 2 * a)
                        emit_vproj(3 + 2 * a)
                    if pr == NPAIR - 1 and j == 0 and a >= 1:
                        # q-block a-1 is now normalized for every pair: its
                        # four output-projection tiles become eligible
                        for tt in range(4 * (a - 1), 4 * a):
                            pending.extend(make_out_closures(tt))
                            out_done[0] = tt + 1
                    if idx + 2 < n_steps:
                        emit_s(idx + 2)
                    emit_pv(idx)
                    if pending:
                        pending.pop(0)()
                while pending:
                    pending.pop(0)()

            # ---- remaining output-projection tiles ----
            for tt in range(out_done[0], S_ // 128):
                for c in make_out_closures(tt):
                    c()

    nc.finalize()
    return nc


def make_host_inputs(x, w_qkv, b_qkv, w_out, b_out, S_=S):
    """Build the 8 per-core input maps (host-side shard/pack/cast)."""
    x = np.asarray(x, dtype=np.float32)
    w_qkv = np.asarray(w_qkv, dtype=np.float32)
    b_qkv = np.asarray(b_qkv, dtype=np.float32)
    w_out = np.asarray(w_out, dtype=np.float32)

    # mask[p, q'] = 0 where q' >= p else -1e30 (q' relative to strip start)
    mask = np.where(
        np.arange(128)[None, :] >= np.arange(128)[:, None], 0.0, NEG
    ).astype(np.float32)

    per_hg = {}
    for hg in range(2):
        wqk = np.empty((128, KO, NPAIR, 2, 128), np.float32)
        bqk = np.empty((128, NPAIR, 2), np.float32)
        for p in range(NPAIR):
            h0, h1 = hg * HPC + 2 * p, hg * HPC + 2 * p + 1
            for r, base in ((0, 0), (1, D)):
                wcols = np.concatenate(
                    [
                        w_qkv[:, base + h0 * HD : base + (h0 + 1) * HD],
                        w_qkv[:, base + h1 * HD : base + (h1 + 1) * HD],
                    ],
                    axis=1,
                )  # [D, 128]
                wqk[:, :, p, r, :] = wcols.reshape(KO, 128, 128).transpose(1, 0, 2)
                bqk[0:64, p, r] = b_qkv[base + h0 * HD : base + (h0 + 1) * HD]
                bqk[64:128, p, r] = b_qkv[base + h1 * HD : base + (h1 + 1) * HD]
        wv = w_qkv[:, 2 * D + hg * 512 : 2 * D + (hg + 1) * 512]  # [D, 512]
        bv = np.broadcast_to(
            b_qkv[2 * D + hg * 512 : 2 * D + (hg + 1) * 512], (128, 512)
        ).copy()
        wout = w_out[hg * 512 : (hg + 1) * 512, :]  # [512, D]
        per_hg[hg] = dict(
            wqk=np.ascontiguousarray(wqk.astype(NPBF16)),
            bqk=bqk,
            wv=np.ascontiguousarray(
                wv.reshape(KO, 128, 512).transpose(1, 0, 2).astype(NPBF16)
            ),
            bv=bv,
            wout=np.ascontiguousarray(
                wout.reshape(NPAIR, 128, D).transpose(1, 0, 2).astype(NPBF16)
            ),
        )

    x_by_b = []
    for b in range(B):
        xt = x[b, :S_].T  # [D, S]
        xtc = xt.reshape(KO, 128, S_ // 512, 512).transpose(2, 1, 0, 3)
        x_by_b.append(np.ascontiguousarray(xtc.astype(NPBF16)))

    in_maps = []
    for c in range(8):
        b, hg = c // 2, c % 2
        m = dict(per_hg[hg])
        m["x"] = x_by_b[b]
        m["mask"] = mask
        in_maps.append(m)
    return in_maps


_NC_CACHE = {}


def _get_nc(S_=S):
    if S_ not in _NC_CACHE:
        _NC_CACHE[S_] = build_nc(S_)
    return _NC_CACHE[S_]


def kernel(x, w_qkv, b_qkv, w_out, b_out):
    x = np.asarray(x, dtype=np.float32)
    b_out = np.asarray(b_out, dtype=np.float32)
    in_maps = make_host_inputs(x, w_qkv, b_qkv, w_out, b_out)
    nc = _get_nc()
    res = run_bass_kernel_spmd(nc, in_maps, list(range(8))).results
    out = np.empty((B, S, D), np.float32)
    for b in range(B):
        out[b] = (
            res[2 * b]["out"].astype(np.float32)
            + res[2 * b + 1]["out"].astype(np.float32)
            + b_out[None, :]
        )
    return out
